# revision 1
# baseline (speedup 1.0000x reference)
"""Trainium2 Bass kernel for nn_Attention_54245436948569.

Full multi-head attention (qkv proj + interleaved RoPE + softmax attention +
out proj) for B=2, N=2048, D=1024, H=16, DH=64, sharded over 8 NeuronCores as
(batch x head-group): core c handles batch c//4 and heads [4*(c%4), 4*(c%4)+4).

Per-core kernel computes a row-parallel partial of the out-projection
([2048, 1024] fp32); the host sums the 4 partials per batch and adds b_out
(the unshard step for row-parallel tensor parallelism).

Matmuls run in bf16 (fp32 PSUM accumulation); softmax runs in fp32 on the
scalar engine with the 1/sqrt(DH) scale folded into exp. The softmax
denominator rides the AV matmul as a ones-column appended to V; the
reciprocal is broadcast across partitions with a K=1 matmul.

Emission order is pipelined so the scalar engine's exp stream (the ~142us
floor) starts early and never starves: k/q projections for pair 0 first,
V-tile production interleaved 1:1 with the first q-block's score tiles, the
rest of qkv hidden in the PE slack of later q-blocks.
"""

import numpy as np
import ml_dtypes

B, N, D = 2, 2048, 1024
H, DH = 16, 64
THETA = 10000.0

BF = ml_dtypes.bfloat16

_CACHE = {}


def _build():
    from contextlib import ExitStack
    import concourse.mybir as mybir
    import concourse.tile as tile
    from concourse import bacc
    from concourse.compiler_utils import get_compiler_flags, set_compiler_flags

    set_compiler_flags([f.replace("--enable-ldw-opt=false", "--enable-ldw-opt=true")
                        for f in get_compiler_flags()])

    FP32 = mybir.dt.float32
    F32R = mybir.dt.float32r
    BF16 = mybir.dt.bfloat16
    AF = mybir.ActivationFunctionType
    MUL = mybir.AluOpType.mult
    ADD = mybir.AluOpType.add

    nc = bacc.Bacc(None, target_bir_lowering=False)

    NT = N // 512            # 4 token 512-blocks
    KT_D = D // 128          # 8 contraction tiles for qkv
    KT_N = N // 128          # 16 k-token tiles for attention
    SCALE = 1.0 / float(np.sqrt(DH))

    with tile.TileContext(nc) as tc:
        with tc.tile_pool(name="dram", bufs=1, space="DRAM") as dram:
            xT_d = dram.tile([NT, 128, KT_D, 512], BF16, kind="ExternalInput", name="xT", uniquify=False)
            wqk_d = dram.tile([128, KT_D, 512], BF16, kind="ExternalInput", name="wqk", uniquify=False)
            wv_d = dram.tile([128, KT_D, 256], BF16, kind="ExternalInput", name="wv", uniquify=False)
            wo_d = dram.tile([128, 2, 1024], BF16, kind="ExternalInput", name="wo", uniquify=False)
            cos_d = dram.tile([128, N], BF16, kind="ExternalInput", name="cos2", uniquify=False)
            sin_d = dram.tile([128, N], BF16, kind="ExternalInput", name="sin2n", uniquify=False)
            out_d = dram.tile([KT_N, 128, D], BF16, kind="ExternalOutput", name="out", uniquify=False)

            ctx = ExitStack()
            const = ctx.enter_context(tc.tile_pool(name="const", bufs=1))
            ropep = ctx.enter_context(tc.tile_pool(name="ropep", bufs=4))
            attnp = ctx.enter_context(tc.tile_pool(name="attnp", bufs=20))
            stkp = ctx.enter_context(tc.tile_pool(name="stkp", bufs=6))
            normp = ctx.enter_context(tc.tile_pool(name="normp", bufs=3))
            outp = ctx.enter_context(tc.tile_pool(name="outp", bufs=3))
            # PSUM budget (8 banks): misc 2x1, scores 2x2, av 2x1
            ps_misc = ctx.enter_context(tc.tile_pool(name="ps_misc", bufs=2, space="PSUM"))
            ps_sc = ctx.enter_context(tc.tile_pool(name="ps_sc", bufs=2, space="PSUM"))
            ps_av = ctx.enter_context(tc.tile_pool(name="ps_av", bufs=2, space="PSUM"))

            # ---- persistent SBUF tensors ----
            wqk_kt = [const.tile([128, 512], BF16, name=f"wqk_{kt}") for kt in range(KT_D)]
            wv = const.tile([128, KT_D, 256], BF16)
            wo = const.tile([128, 2, 1024], BF16)
            cos2 = const.tile([128, N], BF16)
            sin2n = const.tile([128, N], BF16)
            # per (pair, token-block) tiles for fine-grained deps
            q2n = [[const.tile([128, 512], BF16, name=f"q2_{p}_{nt}") for nt in range(NT)] for p in range(2)]
            k2n = [[const.tile([128, 512], BF16, name=f"k2_{p}_{nt}") for nt in range(NT)] for p in range(2)]
            v_t = [const.tile([128, 4, 65], BF16, name=f"v_{tt}") for tt in range(KT_N)]
            ones1 = const.tile([128, 64], BF16)
            ones1f = const.tile([128, 64], FP32)

            # xT is loaded in 512-token chunks so the first q/k projection
            # blocks (which need all 8 D-tiles but only their token slice) can
            # start ~3us after the first chunk lands instead of waiting for
            # the full 4MB.
            xT_nt = [const.tile([128, KT_D, 512], BF16, name=f"xT_{nt}") for nt in range(NT)]
            with nc.named_scope("load"):
                for nt in range(NT):
                    nc.sync.dma_start(out=xT_nt[nt][:], in_=xT_d[nt])
                for kt in range(KT_D):
                    nc.scalar.dma_start(out=wqk_kt[kt][:], in_=wqk_d[:, kt, :])
                nc.scalar.dma_start(out=cos2[:], in_=cos_d[:])
                nc.scalar.dma_start(out=sin2n[:], in_=sin_d[:])
                nc.gpsimd.dma_start(out=wv[:], in_=wv_d[:])
                nc.scalar.dma_start(out=wo[:], in_=wo_d[:])
                nc.vector.memset(ones1[:], 1.0)
                nc.vector.memset(ones1f[:], 1.0)
                # preload the exp table set while DMAs are in flight
                warmup = const.tile([128, 8], FP32)
                nc.scalar.activation(warmup[:], ones1[:, 0:8], AF.Exp, scale=0.125)
                # warm the PE (HAM clock gate) with dummy matmuls during the
                # input DMA wait so the first real matmuls run at 2.4GHz
                wsrc = const.tile([128, 512], BF16)
                nc.vector.memset(wsrc[:], 0.5)
                pw = ps_misc.tile([128, 512], FP32, tag="misc", name="pw")
                for r in range(24):
                    nc.tensor.matmul(pw[:], wsrc[:, 0:128], wsrc[:],
                                     start=(r == 0), stop=(r == 23))
                for tt in range(KT_N):
                    nc.vector.memset(v_t[tt][:, :, 64:65], 1.0)

            pair_mask = []
            for i in range(16):
                pair_mask += [2 * i + 1, 2 * i]

            # ---- building blocks ----
            def qk_mms(m, nt, pqk, kts):
                for kt in kts:
                    nc.tensor.matmul(
                        pqk[:],
                        wqk_kt[kt][:, m * 128:(m + 1) * 128],
                        xT_nt[nt][:, kt, :],
                        start=(kt == 0), stop=(kt == KT_D - 1),
                    )

            def qk_rope(m, nt, pqk, evict_engine):
                dest = (q2n if m < 2 else k2n)[m % 2][nt]
                ts = slice(nt * 512, (nt + 1) * 512)
                qraw = ropep.tile([128, 512], BF16, name="qraw")
                if evict_engine == "scalar":
                    nc.scalar.activation(qraw[:], pqk[:], AF.Copy)
                else:
                    nc.vector.tensor_copy(qraw[:], pqk[:])
                qcos = ropep.tile([128, 512], BF16, name="qcos")
                qsw = ropep.tile([128, 512], BF16, name="qsw")
                tmp = ropep.tile([128, 512], BF16, name="tmp")
                nc.vector.tensor_tensor(out=qcos[:], in0=qraw[:], in1=cos2[:, ts], op=MUL)
                nc.vector.stream_shuffle(qsw[:], qraw[:], pair_mask)
                nc.vector.tensor_tensor(out=tmp[:], in0=qsw[:], in1=sin2n[:, ts], op=MUL)
                nc.vector.tensor_tensor(out=dest[:], in0=qcos[:], in1=tmp[:], op=ADD)

            def qk_proj_nt(m, nt, evict_engine):
                """Project one 512-token block of q or k (m: 0/1=q pair, 2/3=k pair)."""
                pqk = ps_misc.tile([128, 512], FP32, tag="misc", name="pqk")
                qk_mms(m, nt, pqk, range(KT_D))
                qk_rope(m, nt, pqk, evict_engine)

            def qk_proj_pieces(m, nt, evict_engine):
                """Return 4 closures, each emitting 2 of the 8 qkv matmuls (the
                last also emits the RoPE chain), for interleaving into an
                attention block without starving the exp stream."""
                state = {}

                def piece(i):
                    def run():
                        if i == 0:
                            state["pqk"] = ps_misc.tile([128, 512], FP32, tag="misc", name="pqk")
                        qk_mms(m, nt, state["pqk"], range(2 * i, 2 * i + 2))
                        if i == 3:
                            qk_rope(m, nt, state["pqk"], evict_engine)
                    return run
                return [piece(i) for i in range(4)]

            def v_proj(tt):
                pv = ps_misc.tile([128, 512], FP32, tag="misc", name="pv")
                for kt in range(KT_D):
                    nc.tensor.matmul(
                        pv[:, 0:256],
                        xT_nt[tt // 4][:, kt, (tt % 4) * 128:(tt % 4 + 1) * 128],
                        wv[:, kt, :],
                        start=(kt == 0), stop=(kt == KT_D - 1),
                    )
                nc.vector.tensor_copy(v_t[tt][:, :, 0:64], pv[:, 0:256].rearrange("p (h d) -> p h d", d=64))

            def make_block(p, qb):
                """Composable attention block: sc_exp(kt) emits scores+exp,
                av(kt) the AV accumulation, norm() the normalization. Split so
                the driver can hoist the next block's first score tiles ahead
                of this block's AV tail (PE executes strictly in order)."""
                st = {"attnT": {}}

                def sc_exp(kt):
                    with nc.named_scope(f"scores_p{p}_qb{qb}"):
                        pg = ps_sc.tile([128, 2, 512], FP32, tag="pg", name="pg")
                        attnT = attnp.tile([128, 2, 512], BF16, tag="attnT", name="attnT")
                        st["attnT"][kt] = attnT
                        knt, ko = kt // 4, (kt % 4) * 128
                        nc.tensor.matmul(
                            pg[:, 0, :], k2n[p][knt][0:64, ko:ko + 128], q2n[p][qb][0:64, :],
                            start=True, stop=True, tile_position=(0, 0),
                        )
                        nc.tensor.matmul(
                            pg[:, 1, :], k2n[p][knt][64:128, ko:ko + 128], q2n[p][qb][64:128, :],
                            start=True, stop=True, tile_position=(64, 0),
                        )
                        nc.scalar.activation(attnT[:], pg[:], AF.Exp, scale=SCALE)

                def av(kt):
                    with nc.named_scope(f"scores_p{p}_qb{qb}"):
                        if kt == 0:
                            st["pav_a"] = ps_av.tile([128, 512], FP32, tag="pav", name="pav_a")
                            st["pav_b"] = ps_av.tile([128, 512], FP32, tag="pav", name="pav_b")
                        attnT = st["attnT"].pop(kt)
                        nc.tensor.matmul(
                            st["pav_a"][0:65, :], v_t[kt][:, 2 * p, :], attnT[:, 0, :],
                            start=(kt == 0), stop=(kt == KT_N - 1),
                        )
                        nc.tensor.matmul(
                            st["pav_b"][0:65, :], v_t[kt][:, 2 * p + 1, :], attnT[:, 1, :],
                            start=(kt == 0), stop=(kt == KT_N - 1),
                        )

                def norm():
                    pav_a, pav_b = st["pav_a"], st["pav_b"]
                    with nc.named_scope(f"norm_p{p}_qb{qb}"):
                        stkn = stkp.tile([128, 512], BF16, name="stkn")
                        ua = stkp.tile([128, 512], BF16, name="ua", bufs=2)
                        ub = stkp.tile([128, 512], BF16, name="ub", bufs=2)
                        tmpn = stkp.tile([128, 512], BF16, name="tmpn", bufs=2)
                        sums = normp.tile([128, 1024], F32R, name="sums")
                        recipa = normp.tile([128, 512], FP32, name="recipa", bufs=2)
                        recipc = normp.tile([128, 512], FP32, name="recipc", bufs=2)
                        nc.vector.tensor_copy(sums[64:65, 0:512], pav_a[64:65, :])
                        nc.vector.tensor_copy(ua[0:64, :], pav_a[0:64, :])
                        nc.vector.tensor_copy(sums[64:65, 512:1024], pav_b[64:65, :])
                        nc.vector.tensor_copy(ub[0:64, :], pav_b[0:64, :])
                        pbc = ps_misc.tile([128, 512], FP32, tag="misc", name="pbc")
                        nc.tensor.matmul(
                            pbc[0:64, :], ones1f[64:65, :].bitcast(F32R), sums[64:65, 0:512],
                            start=True, stop=True, tile_position=(64, 0),
                        )
                        pbc2 = ps_misc.tile([128, 512], FP32, tag="misc", name="pbc2")
                        nc.tensor.matmul(
                            pbc2[0:64, :], ones1f[64:65, :].bitcast(F32R), sums[64:65, 512:1024],
                            start=True, stop=True, tile_position=(64, 0),
                        )
                        nc.vector.reciprocal_approx_fast(out=recipa[0:64, :], in_=pbc[0:64, :])
                        nc.vector.reciprocal_approx_fast(out=recipc[0:64, :], in_=pbc2[0:64, :])
                        nc.vector.tensor_tensor(out=stkn[0:64, :], in0=ua[0:64, :], in1=recipa[0:64, :], op=MUL)
                        nc.vector.tensor_tensor(out=tmpn[0:64, :], in0=ub[0:64, :], in1=recipc[0:64, :], op=MUL)
                        nc.sync.dma_start(out=stkn[64:128, :], in_=tmpn[0:64, :])
                    return stkn

                return sc_exp, av, norm

            def oproj_piece(qb, qs, dt, state, act_evict=False):
                with nc.named_scope(f"oproj_qb{qb}"):
                    if dt == 0:
                        state["ostg"] = outp.tile([128, 1024], BF16, name="ostg")
                    po = ps_misc.tile([128, 512], FP32, tag="misc", name="po")
                    stacked = state["stacked"]
                    ostg = state["ostg"]
                    for p in range(2):
                        nc.tensor.matmul(
                            po[:],
                            stacked[p][:, qs * 128:(qs + 1) * 128],
                            wo[:, p, dt * 512:(dt + 1) * 512],
                            start=(p == 0), stop=(p == 1),
                        )
                    if act_evict and dt == 1:
                        nc.scalar.activation(ostg[:, dt * 512:(dt + 1) * 512], po[:], AF.Copy)
                    else:
                        nc.vector.tensor_copy(ostg[:, dt * 512:(dt + 1) * 512], po[:])
                    if dt == 1:
                        nc.sync.dma_start(out=out_d[qb * 4 + qs, :, :], in_=ostg[:])

            def oproj_qs(qb, qs, stacked, act_evict=False):
                state = {"stacked": stacked}
                oproj_piece(qb, qs, 0, state, act_evict)
                oproj_piece(qb, qs, 1, state, act_evict)

            # ---- emission schedule ----
            with nc.named_scope("qkv"):
                for nt in range(NT):
                    qk_proj_nt(2, nt, "scalar")   # k pair0
                    qk_proj_nt(0, nt, "scalar")   # q pair0

            # per-block extra work to hide in the ktile loops
            extras = [dict() for _ in range(8)]
            extras[0] = {kt: [(lambda tt=kt: v_proj(tt))] for kt in range(KT_N)}
            for bi, (m3, m1) in zip((1, 2, 3, 4), ((3, 0), (3, 1), (3, 2), (3, 3))):
                pk = qk_proj_pieces(3, m1, "vector")
                pq = qk_proj_pieces(1, m1, "vector")
                d = {}
                for i in range(4):
                    d[2 + i] = [pk[i]]
                    d[8 + i] = [pq[i]]
                extras[bi] = d
            oproj_states = {}
            for bi, qb in zip((5, 6, 7), (0, 1, 2)):
                d = {}
                for qs in range(4):
                    def mk(q, s, dt):
                        def run():
                            if dt == 0 and s == 0:
                                oproj_states[q] = {"stacked": [stacked0[q], stacked1[q]]}
                            oproj_piece(q, s, dt, oproj_states[q])
                        return run
                    d[4 * qs + 1] = [mk(qb, qs, 0)]
                    d[4 * qs + 3] = [mk(qb, qs, 1)]
                extras[bi] = d

            blocks = [(0, 0), (0, 1), (0, 2), (0, 3), (1, 0), (1, 1), (1, 2), (1, 3)]
            units = [make_block(p, qb) for (p, qb) in blocks]
            stacked0 = []
            stacked1 = []
            for bi in range(8):
                sc_exp, av, norm = units[bi]
                for kt in range(KT_N):
                    for fn in extras[bi].get(kt, ()):
                        fn()
                    sc_exp(kt)
                    av(kt)
                stkn = norm()
                (stacked0 if bi < 4 else stacked1).append(stkn)
            for qs in range(4):
                oproj_qs(3, qs, [stacked0[3], stacked1[3]], act_evict=True)

            ctx.close()

    nc.compile()
    return nc


def _host_prep(hidden_states, w_qkv):
    """Per-core input maps (host-side shard + layout prep)."""
    invf = 1.0 / (THETA ** (np.arange(0, DH, 2, dtype=np.float32) / DH))
    t = np.arange(N, dtype=np.float32)
    d_idx = np.arange(128)
    f = invf[(d_idx % 64) // 2]
    ang = t[None, :] * f[:, None]
    cos2 = np.ascontiguousarray(np.cos(ang)).astype(BF)
    sign = np.where(d_idx % 2 == 0, -1.0, 1.0).astype(np.float32)
    sin2n = np.ascontiguousarray(np.sin(ang) * sign[:, None]).astype(BF)

    # [NT, 128, KT_D, 512] partition-major so device DMAs are contiguous
    xT_b = [np.ascontiguousarray(
                hidden_states[b].T.astype(BF).reshape(D // 128, 128, N // 512, 512)
                .transpose(2, 1, 0, 3))
            for b in range(B)]

    in_maps = []
    for c in range(8):
        b, g = c // 4, c % 4
        heads = [4 * g, 4 * g + 1, 4 * g + 2, 4 * g + 3]
        cols = []
        for off in (0, 1024):
            for h in heads:
                cols.append(w_qkv[:, off + h * 64: off + (h + 1) * 64])
        wqk = np.ascontiguousarray(
            np.concatenate(cols, axis=1).astype(BF).reshape(D // 128, 128, 512).transpose(1, 0, 2))
        wv = np.ascontiguousarray(
            np.concatenate([w_qkv[:, 2048 + h * 64: 2048 + (h + 1) * 64] for h in heads],
                           axis=1).astype(BF).reshape(D // 128, 128, 256).transpose(1, 0, 2))
        in_maps.append({
            "xT": xT_b[b],
            "wqk": np.ascontiguousarray(wqk),
            "wv": np.ascontiguousarray(wv),
            "cos2": cos2,
            "sin2n": sin2n,
        })
    return in_maps


def kernel(hidden_states, w_qkv, w_out, b_out, _trace=False, _tmpdir=None):
    hidden_states = np.asarray(hidden_states, dtype=np.float32)
    w_qkv = np.asarray(w_qkv, dtype=np.float32)
    w_out = np.asarray(w_out, dtype=np.float32)
    b_out = np.asarray(b_out, dtype=np.float32)

    from concourse.bass_utils import run_bass_kernel_spmd

    if "nc" not in _CACHE:
        _CACHE["nc"] = _build()
    nc = _CACHE["nc"]

    in_maps = _host_prep(hidden_states, w_qkv)
    for c in range(8):
        g = c % 4
        wo = np.ascontiguousarray(
            w_out[4 * g * 64: 4 * g * 64 + 256, :].astype(BF).reshape(2, 128, 1024).transpose(1, 0, 2))
        in_maps[c]["wo"] = wo

    kwargs = {}
    if _trace:
        kwargs = dict(trace=True, tmpdir=_tmpdir)
    res = run_bass_kernel_spmd(nc, in_maps, core_ids=list(range(8)), **kwargs)

    out = np.zeros((B, N, D), dtype=np.float32)
    for c in range(8):
        out[c // 4] += res.results[c]["out"].reshape(N, D).astype(np.float32)
    out += b_out[None, None, :]
    if _trace:
        _CACHE["last_res"] = res
    return out



# revision 6
# speedup vs baseline: 1.0154x; 1.0154x over previous
"""Trainium2 Bass kernel for nn_Attention_54245436948569.

Full multi-head attention (qkv proj + interleaved RoPE + softmax attention +
out proj) for B=2, N=2048, D=1024, H=16, DH=64, sharded over 8 NeuronCores as
(batch x head-group): core c handles batch c//4 and heads [4*(c%4), 4*(c%4)+4).

Per-core kernel computes a row-parallel partial of the out-projection
([2048, 1024] fp32); the host sums the 4 partials per batch and adds b_out.

The scalar engine's exp stream (128 ACTIVATEs of 1024 elem/partition,
~147us) is the critical path. The emission is a slot-based software
pipeline: slot s emits scores(s) -> av(s-1) -> extras(s), so the next
exp's scores are always immediately behind the current exp, including
across block boundaries (next block's scores are hoisted ahead of the
previous block's AV tail and softmax normalization). The normalization is
split into an urgent PSUM-drain (frees the AV accumulator banks) and a
lazy normalize/broadcast that rides a later slot's slack. Input DMAs are
issued per-kt-chunk on three queues (sync/gpsimd/vector) with the
first-block gate (wqk, xT chunk 0, cos/sin chunk 0, wv pair 0) first so
the exp stream starts ~9us in; the scalar queue carries only the warmup
table-load and the exp stream.
"""

import numpy as np
import ml_dtypes

B, N, D = 2, 2048, 1024
H, DH = 16, 64
THETA = 10000.0

BF = ml_dtypes.bfloat16

_CACHE = {}


def _build():
    from contextlib import ExitStack
    import concourse.mybir as mybir
    import concourse.tile as tile
    from concourse import bacc
    from concourse.compiler_utils import get_compiler_flags, set_compiler_flags

    set_compiler_flags([f.replace("--enable-ldw-opt=false", "--enable-ldw-opt=true")
                        for f in get_compiler_flags()])

    FP32 = mybir.dt.float32
    BF16 = mybir.dt.bfloat16
    AF = mybir.ActivationFunctionType
    MUL = mybir.AluOpType.mult
    ADD = mybir.AluOpType.add

    nc = bacc.Bacc(None, target_bir_lowering=False)

    NT = N // 512            # 4 token 512-blocks
    KT_D = D // 128          # 8 contraction tiles for qkv
    KT_N = N // 128          # 16 k-token tiles for attention
    SCALE = 1.0 / float(np.sqrt(DH))

    with tile.TileContext(nc) as tc:
        with tc.tile_pool(name="dram", bufs=1, space="DRAM") as dram:
            xT_d = dram.tile([NT, 128, KT_D, 512], BF16, kind="ExternalInput", name="xT", uniquify=False)
            wqk_d = dram.tile([128, KT_D, 512], BF16, kind="ExternalInput", name="wqk", uniquify=False)
            wv_d = dram.tile([128, 2, KT_D, 128], BF16, kind="ExternalInput", name="wv", uniquify=False)
            wo_d = dram.tile([128, 2, 1024], BF16, kind="ExternalInput", name="wo", uniquify=False)
            cos_d = dram.tile([128, N], BF16, kind="ExternalInput", name="cos2", uniquify=False)
            sin_d = dram.tile([128, N], BF16, kind="ExternalInput", name="sin2n", uniquify=False)
            out_d = dram.tile([KT_N, 128, D], BF16, kind="ExternalOutput", name="out", uniquify=False)

            ctx = ExitStack()
            const = ctx.enter_context(tc.tile_pool(name="const", bufs=1))
            ropep = ctx.enter_context(tc.tile_pool(name="ropep", bufs=8))
            attnp = ctx.enter_context(tc.tile_pool(name="attnp", bufs=6))
            scrp = ctx.enter_context(tc.tile_pool(name="scrp", bufs=6))
            stknp = ctx.enter_context(tc.tile_pool(name="stknp", bufs=8))
            normp = ctx.enter_context(tc.tile_pool(name="normp", bufs=4))
            outp = ctx.enter_context(tc.tile_pool(name="outp", bufs=3))
            # PSUM budget (8 banks): scores 2x2, av 2x1, qk 1, vm 1
            ps_sc = ctx.enter_context(tc.tile_pool(name="ps_sc", bufs=2, space="PSUM"))
            ps_av = ctx.enter_context(tc.tile_pool(name="ps_av", bufs=2, space="PSUM"))
            ps_qk = ctx.enter_context(tc.tile_pool(name="ps_qk", bufs=1, space="PSUM"))
            ps_vm = ctx.enter_context(tc.tile_pool(name="ps_vm", bufs=1, space="PSUM"))

            # ---- persistent SBUF tensors ----
            wqk_kt = [const.tile([128, 512], BF16, name=f"wqk_{kt}") for kt in range(KT_D)]
            wv = const.tile([128, 2, KT_D, 128], BF16)
            wo = const.tile([128, 2, 1024], BF16)
            cos2 = const.tile([128, N], BF16)
            sin2n = const.tile([128, N], BF16)
            q2n = [[const.tile([128, 512], BF16, name=f"q2_{p}_{nt}") for nt in range(NT)] for p in range(2)]
            k2n = [[const.tile([128, 512], BF16, name=f"k2_{p}_{nt}") for nt in range(NT)] for p in range(2)]
            v_t = [const.tile([128, 4, 65], BF16, name=f"v_{tt}") for tt in range(KT_N)]
            ones1 = const.tile([128, 64], BF16)
            xT_nt = [const.tile([128, KT_D, 512], BF16, name=f"xT_{nt}") for nt in range(NT)]

            # ---- memsets first (vector queue) so the warmup activation and
            # warmup matmuls are gated only on them ----
            with nc.named_scope("load"):
                nc.vector.memset(ones1[:], 1.0)
                wsrc = const.tile([128, 512], BF16)
                nc.vector.memset(wsrc[:], 0.5)
                for tt in range(KT_N):
                    nc.vector.memset(v_t[tt][:, :, 64:65], 1.0)

                # scalar queue: exp table load + warm, then nothing but exps
                warmup = const.tile([128, 8], FP32)
                nc.scalar.activation(warmup[:], ones1[:, 0:8], AF.Exp, scale=0.125)

                # input DMAs: first-exp gate items lead each queue.
                # sync: wqk 0-3 + xT0 kt0-3 interleaved, cos chunk0, xT2, cos rest
                # gpsimd: wqk 4-7 + xT0 kt4-7, sin chunk0, wv pair0, xT3, wv pair1, wo
                # vector: xT1, cos/sin rest
                for kt in range(4):
                    nc.sync.dma_start(out=wqk_kt[kt][:], in_=wqk_d[:, kt, :])
                    nc.sync.dma_start(out=xT_nt[0][:, kt, :], in_=xT_d[0, :, kt, :])
                    nc.gpsimd.dma_start(out=wqk_kt[4 + kt][:], in_=wqk_d[:, 4 + kt, :])
                    nc.gpsimd.dma_start(out=xT_nt[0][:, 4 + kt, :], in_=xT_d[0, :, 4 + kt, :])
                nc.sync.dma_start(out=cos2[:, 0:512], in_=cos_d[:, 0:512])
                nc.gpsimd.dma_start(out=sin2n[:, 0:512], in_=sin_d[:, 0:512])
                nc.gpsimd.dma_start(out=wv[:, 0:1, :, :], in_=wv_d[:, 0:1, :, :])
                for kt in range(4):
                    nc.sync.dma_start(out=xT_nt[1][:, kt, :], in_=xT_d[1, :, kt, :])
                    nc.gpsimd.dma_start(out=xT_nt[1][:, 4 + kt, :], in_=xT_d[1, :, 4 + kt, :])
                nc.sync.dma_start(out=xT_nt[2][:, 0:4, :], in_=xT_d[2, :, 0:4, :])
                nc.gpsimd.dma_start(out=xT_nt[2][:, 4:8, :], in_=xT_d[2, :, 4:8, :])
                nc.sync.dma_start(out=xT_nt[3][:, 0:4, :], in_=xT_d[3, :, 0:4, :])
                nc.gpsimd.dma_start(out=xT_nt[3][:, 4:8, :], in_=xT_d[3, :, 4:8, :])
                nc.gpsimd.dma_start(out=wv[:, 1:2, :, :], in_=wv_d[:, 1:2, :, :])
                nc.sync.dma_start(out=cos2[:, 512:2048], in_=cos_d[:, 512:2048])
                nc.gpsimd.dma_start(out=sin2n[:, 512:2048], in_=sin_d[:, 512:2048])
                nc.gpsimd.dma_start(out=wo[:], in_=wo_d[:])

                # warm the PE (HAM clock gate) during the input DMA wait
                pw = ps_qk.tile([128, 512], FP32, tag="qk", name="pw")
                for r in range(6):
                    nc.tensor.matmul(pw[:], wsrc[:, 0:128], wsrc[:],
                                     start=(r == 0), stop=(r == 5))

            pair_mask = []
            for i in range(16):
                pair_mask += [2 * i + 1, 2 * i]

            # ---- building blocks ----
            def qk_rope(m, nt, pqk, evict_engine):
                dest = (q2n if m < 2 else k2n)[m % 2][nt]
                ts = slice(nt * 512, (nt + 1) * 512)
                qraw = ropep.tile([128, 512], BF16, name="qraw")
                if evict_engine == "scalar":
                    nc.scalar.activation(qraw[:], pqk[:], AF.Copy)
                else:
                    nc.vector.tensor_copy(qraw[:], pqk[:])
                qcos = ropep.tile([128, 512], BF16, name="qcos")
                qsw = ropep.tile([128, 512], BF16, name="qsw")
                tmp = ropep.tile([128, 512], BF16, name="tmp")
                nc.vector.tensor_tensor(out=qcos[:], in0=qraw[:], in1=cos2[:, ts], op=MUL)
                nc.vector.stream_shuffle(qsw[:], qraw[:], pair_mask)
                nc.vector.tensor_tensor(out=tmp[:], in0=qsw[:], in1=sin2n[:, ts], op=MUL)
                nc.vector.tensor_tensor(out=dest[:], in0=qcos[:], in1=tmp[:], op=ADD)

            def qk_proj_nt(m, nt, evict_engine):
                pqk = ps_qk.tile([128, 512], FP32, tag="qk", name="pqk")
                for kt in range(KT_D):
                    nc.tensor.matmul(
                        pqk[:], wqk_kt[kt][:, m * 128:(m + 1) * 128], xT_nt[nt][:, kt, :],
                        start=(kt == 0), stop=(kt == KT_D - 1),
                    )
                qk_rope(m, nt, pqk, evict_engine)

            def qk_proj_pieces2(m, nt, evict_engine):
                """Two closures, each emitting 4 of the 8 qkv matmuls (the
                second also emits the RoPE chain)."""
                state = {}

                def piece(i):
                    def run():
                        if i == 0:
                            state["pqk"] = ps_qk.tile([128, 512], FP32, tag="qk", name="pqk")
                        for kt in range(4 * i, 4 * i + 4):
                            nc.tensor.matmul(
                                state["pqk"][:], wqk_kt[kt][:, m * 128:(m + 1) * 128],
                                xT_nt[nt][:, kt, :],
                                start=(kt == 0), stop=(kt == KT_D - 1),
                            )
                        if i == 1:
                            qk_rope(m, nt, state["pqk"], evict_engine)
                    return run
                return [piece(0), piece(1)]

            def v_half(tt, p):
                """Produce heads (2p, 2p+1) of v_t[tt] (pair p)."""
                def run():
                    pv = ps_vm.tile([128, 512], FP32, tag="vm", name="pv")
                    for kt in range(KT_D):
                        nc.tensor.matmul(
                            pv[:, 0:128],
                            xT_nt[tt // 4][:, kt, (tt % 4) * 128:(tt % 4 + 1) * 128],
                            wv[:, p, kt, :],
                            start=(kt == 0), stop=(kt == KT_D - 1),
                        )
                    nc.vector.tensor_copy(
                        v_t[tt][:, 2 * p:2 * p + 2, 0:64],
                        pv[:, 0:128].rearrange("p (h d) -> p h d", d=64))
                return run

            def make_block(p, qb):
                st = {"attnT": {}}

                def sc_exp(kt):
                    with nc.named_scope(f"scores_p{p}_qb{qb}"):
                        pg = ps_sc.tile([128, 2, 512], FP32, tag="pg", name="pg")
                        attnT = attnp.tile([128, 2, 512], BF16, tag="attnT", name="attnT")
                        st["attnT"][kt] = attnT
                        knt, ko = kt // 4, (kt % 4) * 128
                        nc.tensor.matmul(
                            pg[:, 0, :], k2n[p][knt][0:64, ko:ko + 128], q2n[p][qb][0:64, :],
                            start=True, stop=True, tile_position=(0, 0),
                        )
                        nc.tensor.matmul(
                            pg[:, 1, :], k2n[p][knt][64:128, ko:ko + 128], q2n[p][qb][64:128, :],
                            start=True, stop=True, tile_position=(64, 0),
                        )
                        nc.scalar.activation(attnT[:], pg[:], AF.Exp, scale=SCALE)

                def av(kt):
                    with nc.named_scope(f"scores_p{p}_qb{qb}"):
                        if kt == 0:
                            st["pav_a"] = ps_av.tile([128, 512], FP32, tag="pav", name="pav_a")
                            st["pav_b"] = ps_av.tile([128, 512], FP32, tag="pav", name="pav_b")
                        attnT = st["attnT"].pop(kt)
                        nc.tensor.matmul(
                            st["pav_a"][0:65, :], v_t[kt][:, 2 * p, :], attnT[:, 0, :],
                            start=(kt == 0), stop=(kt == KT_N - 1),
                        )
                        nc.tensor.matmul(
                            st["pav_b"][0:65, :], v_t[kt][:, 2 * p + 1, :], attnT[:, 1, :],
                            start=(kt == 0), stop=(kt == KT_N - 1),
                        )

                def norm_drain():
                    # urgent: drain AV accumulators (incl. denominator row 64)
                    # to SBUF bf16 so the next block's AV can claim the banks
                    with nc.named_scope(f"norm_p{p}_qb{qb}"):
                        ua = scrp.tile([128, 512], BF16, name="ua")
                        ub = scrp.tile([128, 512], BF16, name="ub")
                        nc.vector.tensor_copy(ua[0:65, :], st["pav_a"][0:65, :])
                        nc.vector.tensor_copy(ub[0:65, :], st["pav_b"][0:65, :])
                        st["ua"], st["ub"] = ua, ub

                def norm_fin():
                    with nc.named_scope(f"norm_p{p}_qb{qb}"):
                        ua, ub = st["ua"], st["ub"]
                        stkn = stknp.tile([128, 512], BF16, name="stkn")
                        tmpn = scrp.tile([128, 512], BF16, name="tmpn")
                        recipa = normp.tile([128, 512], FP32, name="recipa")
                        recipc = normp.tile([128, 512], FP32, name="recipc")
                        pbc = ps_vm.tile([128, 512], FP32, tag="vm", name="pbc")
                        nc.tensor.matmul(
                            pbc[0:64, :], ones1[64:65, :], ua[64:65, :],
                            start=True, stop=True, tile_position=(64, 0),
                        )
                        nc.vector.reciprocal_approx_fast(out=recipa[0:64, :], in_=pbc[0:64, :])
                        pbc2 = ps_vm.tile([128, 512], FP32, tag="vm", name="pbc2")
                        nc.tensor.matmul(
                            pbc2[0:64, :], ones1[64:65, :], ub[64:65, :],
                            start=True, stop=True, tile_position=(64, 0),
                        )
                        nc.vector.reciprocal_approx_fast(out=recipc[0:64, :], in_=pbc2[0:64, :])
                        nc.vector.tensor_tensor(out=stkn[0:64, :], in0=ua[0:64, :], in1=recipa[0:64, :], op=MUL)
                        nc.vector.tensor_tensor(out=tmpn[0:64, :], in0=ub[0:64, :], in1=recipc[0:64, :], op=MUL)
                        nc.sync.dma_start(out=stkn[64:128, :], in_=tmpn[0:64, :])
                    return stkn

                return sc_exp, av, norm_drain, norm_fin

            stacked = {}   # (p, qb) -> stkn tile

            def oproj_piece(qb, qs, dt, state, evict="vector"):
                with nc.named_scope(f"oproj_qb{qb}"):
                    if dt == 0:
                        state["ostg"] = outp.tile([128, 1024], BF16, name="ostg")
                    po = ps_vm.tile([128, 512], FP32, tag="vm", name="po")
                    ostg = state["ostg"]
                    for p in range(2):
                        nc.tensor.matmul(
                            po[:],
                            stacked[(p, qb)][:, qs * 128:(qs + 1) * 128],
                            wo[:, p, dt * 512:(dt + 1) * 512],
                            start=(p == 0), stop=(p == 1),
                        )
                    if evict == "scalar":
                        nc.scalar.activation(ostg[:, dt * 512:(dt + 1) * 512], po[:], AF.Copy)
                    else:
                        nc.vector.tensor_copy(ostg[:, dt * 512:(dt + 1) * 512], po[:])
                    if dt == 1:
                        eng = nc.sync if qs % 2 == 0 else nc.gpsimd
                        eng.dma_start(out=out_d[qb * 4 + qs, :, :], in_=ostg[:])

            # ---- head: first projections (k pair0 nt0, q pair0 nt0) ----
            with nc.named_scope("qkv"):
                qk_proj_nt(2, 0, "scalar")
                qk_proj_nt(0, 0, "scalar")

            # ---- extras map: slot -> list of closures ----
            extras = {s: [] for s in range(128)}

            def add(s, fn):
                extras[s].append(fn)

            # block 0: v pair0 halves + remaining pair0 k/q projections
            for tt in range(KT_N):
                add(tt, v_half(tt, 0))
            pcs = qk_proj_pieces2(2, 1, "vector"); add(0, pcs[0]); add(1, pcs[1])
            pcs = qk_proj_pieces2(2, 2, "vector"); add(2, pcs[0]); add(3, pcs[1])
            pcs = qk_proj_pieces2(2, 3, "vector"); add(4, pcs[0]); add(5, pcs[1])
            pcs = qk_proj_pieces2(0, 1, "vector"); add(11, pcs[0]); add(12, pcs[1])
            # blocks 1-4: remaining q pair0, all pair1 k/q, v pair1 halves
            def add_proj(s0, m, nt):
                p0, p1 = qk_proj_pieces2(m, nt, "vector")
                add(s0, p0); add(s0 + 1, p1)
            add_proj(18, 0, 2)          # q0n2, needed slot 32
            add_proj(22, 3, 0)          # k1n0, needed slot 64
            for tt in range(5):
                add(26 + tt, v_half(tt, 1))
            add_proj(34, 0, 3)          # q0n3, needed slot 48
            add_proj(38, 3, 1)          # k1n1
            for tt in range(5, 10):
                add(42 + (tt - 5), v_half(tt, 1))
            add_proj(50, 3, 2)          # k1n2
            add_proj(54, 3, 3)          # k1n3
            add_proj(58, 1, 0)          # q1n0, needed slot 64
            for tt in range(10, 13):
                add(61 + (tt - 10), v_half(tt, 1))
            for tt in range(13, 16):
                add(66 + (tt - 13), v_half(tt, 1))
            add_proj(70, 1, 1)          # q1n1, needed slot 80
            add_proj(90, 1, 2)          # q1n2, needed slot 96
            add_proj(106, 1, 3)         # q1n3, needed slot 112

            # oproj for qb 0,1,2 embedded in blocks 5,6,7
            for blk, qb in ((5, 0), (6, 1), (7, 2)):
                st_o = {}
                for qs in range(4):
                    for dt in range(2):
                        def mk(q=qb, s=qs, d=dt, stx=st_o):
                            def run():
                                oproj_piece(q, s, d, stx)
                            return run
                        add(16 * blk + 2 + 2 * qs + dt, mk())

            # ---- slot pipeline ----
            blocks = [(0, 0), (0, 1), (0, 2), (0, 3), (1, 0), (1, 1), (1, 2), (1, 3)]
            units = [make_block(p, qb) for (p, qb) in blocks]
            pend = None
            for s in range(128):
                bi, kt = s // 16, s % 16
                units[bi][0](kt)                       # sc_exp
                if pend is not None:
                    units[pend[0]][1](pend[1])         # av of previous slot
                pend = (bi, kt)
                if kt == 0 and bi > 0:
                    units[bi - 1][2]()                 # norm_drain prev block
                if kt == 1 and bi > 0:
                    pprev, qprev = blocks[bi - 1]
                    stacked[(pprev, qprev)] = units[bi - 1][3]()   # norm_fin
                for fn in extras[s]:
                    fn()

            # ---- tail ----
            units[7][1](15)
            units[7][2]()
            stacked[(1, 3)] = units[7][3]()
            st_o = {}
            for qs in range(4):
                for dt in range(2):
                    oproj_piece(3, qs, dt, st_o, evict=("scalar" if (2 * qs + dt) % 2 else "vector"))

            ctx.close()

    nc.compile()
    return nc


def _host_prep(hidden_states, w_qkv):
    """Per-core input maps (host-side shard + layout prep)."""
    invf = 1.0 / (THETA ** (np.arange(0, DH, 2, dtype=np.float32) / DH))
    t = np.arange(N, dtype=np.float32)
    d_idx = np.arange(128)
    f = invf[(d_idx % 64) // 2]
    ang = t[None, :] * f[:, None]
    cos2 = np.ascontiguousarray(np.cos(ang)).astype(BF)
    sign = np.where(d_idx % 2 == 0, -1.0, 1.0).astype(np.float32)
    sin2n = np.ascontiguousarray(np.sin(ang) * sign[:, None]).astype(BF)

    # [NT, 128, KT_D, 512] partition-major so device DMAs are contiguous
    xT_b = [np.ascontiguousarray(
                hidden_states[b].T.astype(BF).reshape(D // 128, 128, N // 512, 512)
                .transpose(2, 1, 0, 3))
            for b in range(B)]

    in_maps = []
    for c in range(8):
        b, g = c // 4, c % 4
        heads = [4 * g, 4 * g + 1, 4 * g + 2, 4 * g + 3]
        cols = []
        for off in (0, 1024):
            for h in heads:
                cols.append(w_qkv[:, off + h * 64: off + (h + 1) * 64])
        wqk = np.ascontiguousarray(
            np.concatenate(cols, axis=1).astype(BF).reshape(D // 128, 128, 512).transpose(1, 0, 2))
        # wv pair-major: [128, 2, KT_D, 128]
        wv = np.ascontiguousarray(
            np.concatenate([w_qkv[:, 2048 + h * 64: 2048 + (h + 1) * 64] for h in heads],
                           axis=1).astype(BF).reshape(D // 128, 128, 2, 128)
            .transpose(1, 2, 0, 3))
        in_maps.append({
            "xT": xT_b[b],
            "wqk": np.ascontiguousarray(wqk),
            "wv": np.ascontiguousarray(wv),
            "cos2": cos2,
            "sin2n": sin2n,
        })
    return in_maps


def kernel(hidden_states, w_qkv, w_out, b_out, _trace=False, _tmpdir=None):
    hidden_states = np.asarray(hidden_states, dtype=np.float32)
    w_qkv = np.asarray(w_qkv, dtype=np.float32)
    w_out = np.asarray(w_out, dtype=np.float32)
    b_out = np.asarray(b_out, dtype=np.float32)

    from concourse.bass_utils import run_bass_kernel_spmd

    if "nc" not in _CACHE:
        _CACHE["nc"] = _build()
    nc = _CACHE["nc"]

    in_maps = _host_prep(hidden_states, w_qkv)
    for c in range(8):
        g = c % 4
        wo = np.ascontiguousarray(
            w_out[4 * g * 64: 4 * g * 64 + 256, :].astype(BF).reshape(2, 128, 1024).transpose(1, 0, 2))
        in_maps[c]["wo"] = wo

    kwargs = {}
    if _trace:
        kwargs = dict(trace=True, tmpdir=_tmpdir)
    res = run_bass_kernel_spmd(nc, in_maps, core_ids=list(range(8)), **kwargs)

    out = np.zeros((B, N, D), dtype=np.float32)
    for c in range(8):
        out[c // 4] += res.results[c]["out"].reshape(N, D).astype(np.float32)
    out += b_out[None, None, :]
    if _trace:
        _CACHE["last_res"] = res
    return out


# revision 9
# speedup vs baseline: 1.0694x; 1.0533x over previous
"""Trainium2 Bass kernel for nn_Attention_54245436948569.

Full multi-head attention (qkv proj + interleaved RoPE + softmax attention +
out proj) for B=2, N=2048, D=1024, H=16, DH=64, sharded over 8 NeuronCores as
(batch x head-group): core c handles batch c//4 and heads [4*(c%4), 4*(c%4)+4).

Per-core kernel computes a row-parallel partial of the out-projection
([2048, 1024] fp32); the host sums the 4 partials per batch and adds b_out.

The scalar engine's exp stream (128 ACTIVATEs of 1024 elem/partition,
~147us) is the critical path. The emission is a slot-based software
pipeline: slot s emits scores(s) -> av(s-1) -> extras(s), so the next
exp's scores are always immediately behind the current exp, including
across block boundaries (next block's scores are hoisted ahead of the
previous block's AV tail and softmax normalization). The normalization is
split into an urgent PSUM-drain (frees the AV accumulator banks) and a
lazy normalize/broadcast that rides a later slot's slack. Input DMAs are
issued per-kt-chunk on three queues (sync/gpsimd/vector) with the
first-block gate (wqk, xT chunk 0, cos/sin chunk 0, wv pair 0) first so
the exp stream starts ~9us in; the scalar queue carries only the warmup
table-load and the exp stream.
"""

import numpy as np
import ml_dtypes

B, N, D = 2, 2048, 1024
H, DH = 16, 64
THETA = 10000.0

BF = ml_dtypes.bfloat16

_CACHE = {}


def _build():
    from contextlib import ExitStack
    import concourse.mybir as mybir
    import concourse.tile as tile
    from concourse import bacc
    from concourse.compiler_utils import get_compiler_flags, set_compiler_flags

    set_compiler_flags([f.replace("--enable-ldw-opt=false", "--enable-ldw-opt=true")
                        for f in get_compiler_flags()])

    FP32 = mybir.dt.float32
    BF16 = mybir.dt.bfloat16
    AF = mybir.ActivationFunctionType
    MUL = mybir.AluOpType.mult
    ADD = mybir.AluOpType.add

    nc = bacc.Bacc(None, target_bir_lowering=False)

    NT = N // 512            # 4 token 512-blocks
    KT_D = D // 128          # 8 contraction tiles for qkv
    KT_N = N // 128          # 16 k-token tiles for attention
    SCALE = 1.0 / float(np.sqrt(DH))

    with tile.TileContext(nc) as tc:
        with tc.tile_pool(name="dram", bufs=1, space="DRAM") as dram:
            xT_d = dram.tile([NT, 128, KT_D, 512], BF16, kind="ExternalInput", name="xT", uniquify=False)
            wqk_d = dram.tile([128, KT_D, 512], BF16, kind="ExternalInput", name="wqk", uniquify=False)
            wv_d = dram.tile([128, 2, KT_D, 128], BF16, kind="ExternalInput", name="wv", uniquify=False)
            wo_d = dram.tile([128, 2, 1024], BF16, kind="ExternalInput", name="wo", uniquify=False)
            cos_d = dram.tile([128, N], BF16, kind="ExternalInput", name="cos2", uniquify=False)
            sin_d = dram.tile([128, N], BF16, kind="ExternalInput", name="sin2n", uniquify=False)
            out_d = dram.tile([KT_N, 128, D], BF16, kind="ExternalOutput", name="out", uniquify=False)

            ctx = ExitStack()
            const = ctx.enter_context(tc.tile_pool(name="const", bufs=1))
            ropep = ctx.enter_context(tc.tile_pool(name="ropep", bufs=8))
            attnp = ctx.enter_context(tc.tile_pool(name="attnp", bufs=6))
            scrp = ctx.enter_context(tc.tile_pool(name="scrp", bufs=6))
            stknp = ctx.enter_context(tc.tile_pool(name="stknp", bufs=8))
            normp = ctx.enter_context(tc.tile_pool(name="normp", bufs=4))
            outp = ctx.enter_context(tc.tile_pool(name="outp", bufs=3))
            # PSUM budget (8 banks): scores 2x2, av 2x1, qk 1, vm 1
            ps_sc = ctx.enter_context(tc.tile_pool(name="ps_sc", bufs=2, space="PSUM"))
            ps_av = ctx.enter_context(tc.tile_pool(name="ps_av", bufs=2, space="PSUM"))
            ps_qk = ctx.enter_context(tc.tile_pool(name="ps_qk", bufs=1, space="PSUM"))
            ps_vm = ctx.enter_context(tc.tile_pool(name="ps_vm", bufs=1, space="PSUM"))

            # ---- persistent SBUF tensors ----
            wqk4_0 = const.tile([128, 4, 512], BF16, name="wqk4_0")
            wqk4_1 = const.tile([128, 4, 512], BF16, name="wqk4_1")
            wqk_kt = [(wqk4_0 if kt < 4 else wqk4_1)[:, kt % 4, :] for kt in range(KT_D)]
            wv = const.tile([128, 2, KT_D, 128], BF16)
            wo = const.tile([128, 2, 1024], BF16)
            cos2 = const.tile([128, N], BF16)
            sin2n = const.tile([128, N], BF16)
            q2n = [[const.tile([128, 512], BF16, name=f"q2_{p}_{nt}") for nt in range(NT)] for p in range(2)]
            k2n = [[const.tile([128, 512], BF16, name=f"k2_{p}_{nt}") for nt in range(NT)] for p in range(2)]
            v_t = [const.tile([128, 4, 65], BF16, name=f"v_{tt}") for tt in range(KT_N)]
            ones1 = const.tile([128, 64], BF16)
            xT_nt = [const.tile([128, KT_D, 512], BF16, name=f"xT_{nt}") for nt in range(NT)]

            # ---- memsets first (vector queue) so the warmup activation and
            # warmup matmuls are gated only on them ----
            with nc.named_scope("load"):
                nc.vector.memset(ones1[:], 1.0)
                wsrc = const.tile([128, 512], BF16)
                nc.vector.memset(wsrc[:], 0.5)
                for tt in range(KT_N):
                    nc.vector.memset(v_t[tt][:, :, 64:65], 1.0)

                # scalar queue: exp table load + warm, then nothing but exps
                warmup = const.tile([128, 8], FP32)
                nc.scalar.activation(warmup[:], ones1[:, 0:8], AF.Exp, scale=0.125)

                # input DMAs: first-exp gate items lead each queue.
                # sync: wqk 0-3 + xT0 kt0-3 interleaved, cos chunk0, xT2, cos rest
                # gpsimd: wqk 4-7 + xT0 kt4-7, sin chunk0, wv pair0, xT3, wv pair1, wo
                # vector: xT1, cos/sin rest
                # sync: wqk kt0-3, cos0, xT0 kt0-3, xT1 kt0-3, cos rest, xT2a, xT3a
                # gpsimd: wqk kt4-7, sin0, xT0 kt4-7, wv p0, xT1 kt4-7, wv p1, xT2b, sin rest, xT3b, wo
                nc.sync.dma_start(out=wqk4_0[:], in_=wqk_d[:, 0:4, :])
                nc.gpsimd.dma_start(out=wqk4_1[:], in_=wqk_d[:, 4:8, :])
                nc.sync.dma_start(out=cos2[:, 0:512], in_=cos_d[:, 0:512])
                nc.gpsimd.dma_start(out=sin2n[:, 0:512], in_=sin_d[:, 0:512])
                nc.sync.dma_start(out=xT_nt[0][:, 0:4, :], in_=xT_d[0, :, 0:4, :])
                nc.gpsimd.dma_start(out=xT_nt[0][:, 4:8, :], in_=xT_d[0, :, 4:8, :])
                nc.gpsimd.dma_start(out=wv[:, 0:1, :, :], in_=wv_d[:, 0:1, :, :])
                nc.sync.dma_start(out=xT_nt[1][:, 0:4, :], in_=xT_d[1, :, 0:4, :])
                nc.gpsimd.dma_start(out=xT_nt[1][:, 4:8, :], in_=xT_d[1, :, 4:8, :])
                nc.sync.dma_start(out=cos2[:, 512:2048], in_=cos_d[:, 512:2048])
                nc.gpsimd.dma_start(out=wv[:, 1:2, :, :], in_=wv_d[:, 1:2, :, :])
                nc.sync.dma_start(out=xT_nt[2][:, 0:4, :], in_=xT_d[2, :, 0:4, :])
                nc.gpsimd.dma_start(out=xT_nt[2][:, 4:8, :], in_=xT_d[2, :, 4:8, :])
                nc.sync.dma_start(out=xT_nt[3][:, 0:4, :], in_=xT_d[3, :, 0:4, :])
                nc.gpsimd.dma_start(out=sin2n[:, 512:2048], in_=sin_d[:, 512:2048])
                nc.gpsimd.dma_start(out=xT_nt[3][:, 4:8, :], in_=xT_d[3, :, 4:8, :])
                nc.gpsimd.dma_start(out=wo[:], in_=wo_d[:])

                # warm the PE (HAM clock gate) through the input DMA wait so
                # the first projections run at 2.4GHz (~11us of dummy work)
                pw = ps_qk.tile([128, 512], FP32, tag="qk", name="pw")
                for r in range(36):
                    nc.tensor.matmul(pw[:], wsrc[:, 0:128], wsrc[:],
                                     start=(r == 0), stop=(r == 35))

            pair_mask = []
            for i in range(16):
                pair_mask += [2 * i + 1, 2 * i]

            # ---- building blocks ----
            def qk_rope(m, nt, pqk, evict_engine):
                dest = (q2n if m < 2 else k2n)[m % 2][nt]
                ts = slice(nt * 512, (nt + 1) * 512)
                qraw = ropep.tile([128, 512], BF16, name="qraw")
                if evict_engine == "scalar":
                    nc.scalar.activation(qraw[:], pqk[:], AF.Copy)
                else:
                    nc.vector.tensor_copy(qraw[:], pqk[:])
                qcos = ropep.tile([128, 512], BF16, name="qcos")
                qsw = ropep.tile([128, 512], BF16, name="qsw")
                tmp = ropep.tile([128, 512], BF16, name="tmp")
                nc.vector.tensor_tensor(out=qcos[:], in0=qraw[:], in1=cos2[:, ts], op=MUL)
                nc.vector.stream_shuffle(qsw[:], qraw[:], pair_mask)
                nc.vector.tensor_tensor(out=tmp[:], in0=qsw[:], in1=sin2n[:, ts], op=MUL)
                nc.vector.tensor_tensor(out=dest[:], in0=qcos[:], in1=tmp[:], op=ADD)

            def qk_proj_nt(m, nt, evict_engine):
                pqk = ps_qk.tile([128, 512], FP32, tag="qk", name="pqk")
                for kt in range(KT_D):
                    nc.tensor.matmul(
                        pqk[:], wqk_kt[kt][:, m * 128:(m + 1) * 128], xT_nt[nt][:, kt, :],
                        start=(kt == 0), stop=(kt == KT_D - 1),
                    )
                qk_rope(m, nt, pqk, evict_engine)

            def qk_proj_pieces2(m, nt, evict_engine):
                """Two closures, each emitting 4 of the 8 qkv matmuls (the
                second also emits the RoPE chain)."""
                state = {}

                def piece(i):
                    def run():
                        if i == 0:
                            state["pqk"] = ps_qk.tile([128, 512], FP32, tag="qk", name="pqk")
                        for kt in range(4 * i, 4 * i + 4):
                            nc.tensor.matmul(
                                state["pqk"][:], wqk_kt[kt][:, m * 128:(m + 1) * 128],
                                xT_nt[nt][:, kt, :],
                                start=(kt == 0), stop=(kt == KT_D - 1),
                            )
                        if i == 1:
                            qk_rope(m, nt, state["pqk"], evict_engine)
                    return run
                return [piece(0), piece(1)]

            def v_half(tt, p):
                """Produce heads (2p, 2p+1) of v_t[tt] (pair p)."""
                def run():
                    pv = ps_vm.tile([128, 512], FP32, tag="vm", name="pv")
                    for kt in range(KT_D):
                        nc.tensor.matmul(
                            pv[:, 0:128],
                            xT_nt[tt // 4][:, kt, (tt % 4) * 128:(tt % 4 + 1) * 128],
                            wv[:, p, kt, :],
                            start=(kt == 0), stop=(kt == KT_D - 1),
                        )
                    nc.vector.tensor_copy(
                        v_t[tt][:, 2 * p:2 * p + 2, 0:64],
                        pv[:, 0:128].rearrange("p (h d) -> p h d", d=64))
                return run

            def make_block(p, qb):
                st = {"attnT": {}}

                def sc_exp(kt):
                    with nc.named_scope(f"scores_p{p}_qb{qb}"):
                        pg = ps_sc.tile([128, 2, 512], FP32, tag="pg", name="pg")
                        attnT = attnp.tile([128, 2, 512], BF16, tag="attnT", name="attnT")
                        st["attnT"][kt] = attnT
                        knt, ko = kt // 4, (kt % 4) * 128
                        nc.tensor.matmul(
                            pg[:, 0, :], k2n[p][knt][0:64, ko:ko + 128], q2n[p][qb][0:64, :],
                            start=True, stop=True, tile_position=(0, 0),
                        )
                        nc.tensor.matmul(
                            pg[:, 1, :], k2n[p][knt][64:128, ko:ko + 128], q2n[p][qb][64:128, :],
                            start=True, stop=True, tile_position=(64, 0),
                        )
                        nc.scalar.activation(attnT[:], pg[:], AF.Exp, scale=SCALE)

                def av(kt):
                    with nc.named_scope(f"scores_p{p}_qb{qb}"):
                        if kt == 0:
                            st["pav_a"] = ps_av.tile([128, 512], FP32, tag="pav", name="pav_a")
                            st["pav_b"] = ps_av.tile([128, 512], FP32, tag="pav", name="pav_b")
                        attnT = st["attnT"].pop(kt)
                        nc.tensor.matmul(
                            st["pav_a"][0:65, :], v_t[kt][:, 2 * p, :], attnT[:, 0, :],
                            start=(kt == 0), stop=(kt == KT_N - 1),
                        )
                        nc.tensor.matmul(
                            st["pav_b"][0:65, :], v_t[kt][:, 2 * p + 1, :], attnT[:, 1, :],
                            start=(kt == 0), stop=(kt == KT_N - 1),
                        )

                def norm_drain():
                    # urgent: drain AV accumulators (incl. denominator row 64)
                    # to SBUF bf16 so the next block's AV can claim the banks
                    with nc.named_scope(f"norm_p{p}_qb{qb}"):
                        ua = scrp.tile([128, 512], BF16, name="ua")
                        ub = scrp.tile([128, 512], BF16, name="ub")
                        nc.vector.tensor_copy(ua[0:65, :], st["pav_a"][0:65, :])
                        nc.vector.tensor_copy(ub[0:65, :], st["pav_b"][0:65, :])
                        st["ua"], st["ub"] = ua, ub

                def norm_fin():
                    with nc.named_scope(f"norm_p{p}_qb{qb}"):
                        ua, ub = st["ua"], st["ub"]
                        stkn = stknp.tile([128, 512], BF16, name="stkn")
                        tmpn = scrp.tile([128, 512], BF16, name="tmpn")
                        recipa = normp.tile([128, 512], FP32, name="recipa")
                        recipc = normp.tile([128, 512], FP32, name="recipc")
                        pbc = ps_vm.tile([128, 512], FP32, tag="vm", name="pbc")
                        nc.tensor.matmul(
                            pbc[0:64, :], ones1[64:65, :], ua[64:65, :],
                            start=True, stop=True, tile_position=(64, 0),
                        )
                        nc.vector.reciprocal_approx_fast(out=recipa[0:64, :], in_=pbc[0:64, :])
                        pbc2 = ps_qk.tile([128, 512], FP32, tag="qk", name="pbc2")
                        nc.tensor.matmul(
                            pbc2[0:64, :], ones1[64:65, :], ub[64:65, :],
                            start=True, stop=True, tile_position=(64, 0),
                        )
                        nc.vector.reciprocal_approx_fast(out=recipc[0:64, :], in_=pbc2[0:64, :])
                        nc.vector.tensor_tensor(out=stkn[0:64, :], in0=ua[0:64, :], in1=recipa[0:64, :], op=MUL)
                        nc.vector.tensor_tensor(out=tmpn[0:64, :], in0=ub[0:64, :], in1=recipc[0:64, :], op=MUL)
                        nc.sync.dma_start(out=stkn[64:128, :], in_=tmpn[0:64, :])
                    return stkn

                return sc_exp, av, norm_drain, norm_fin

            stacked = {}   # (p, qb) -> stkn tile

            def oproj_piece(qb, qs, dt, state, evict="vector", po_pool=None):
                with nc.named_scope(f"oproj_qb{qb}"):
                    if dt == 0:
                        state["ostg"] = outp.tile([128, 1024], BF16, name="ostg")
                    pool = po_pool if po_pool is not None else ps_vm
                    tg = {id(ps_vm): "vm", id(ps_qk): "qk", id(ps_av): "pav"}[id(pool)]
                    po = pool.tile([128, 512], FP32, tag=tg, name="po")
                    ostg = state["ostg"]
                    for p in range(2):
                        nc.tensor.matmul(
                            po[:],
                            stacked[(p, qb)][:, qs * 128:(qs + 1) * 128],
                            wo[:, p, dt * 512:(dt + 1) * 512],
                            start=(p == 0), stop=(p == 1),
                        )
                    if evict == "scalar":
                        nc.scalar.activation(ostg[:, dt * 512:(dt + 1) * 512], po[:], AF.Copy)
                    else:
                        nc.vector.tensor_copy(ostg[:, dt * 512:(dt + 1) * 512], po[:])
                    if dt == 1:
                        eng = nc.sync if qs % 2 == 0 else nc.gpsimd
                        eng.dma_start(out=out_d[qb * 4 + qs, :, :], in_=ostg[:])

            # ---- head: first projections (k0/q0 nt0 + k0 nt1, evicts on
            # the still-idle scalar queue) ----
            with nc.named_scope("qkv"):
                qk_proj_nt(2, 0, "scalar")
                qk_proj_nt(0, 0, "scalar")
                qk_proj_nt(2, 1, "scalar")

            # ---- extras map: slot -> list of closures ----
            extras = {s: [] for s in range(128)}

            def add(s, fn):
                extras[s].append(fn)

            def add_proj(s0, m, nt):
                p0, p1 = qk_proj_pieces2(m, nt, "vector")
                add(s0, p0); add(s0 + 1, p1)

            # block 0: v pair0 halves + remaining pair0 k/q projections
            for tt in range(KT_N):
                add(tt, v_half(tt, 0))
            add_proj(0, 2, 2)           # k0n2, needed slot 8
            add_proj(4, 2, 3)           # k0n3, needed slot 12
            add_proj(8, 0, 1)           # q0n1, needed slot 16
            # blocks 1-4: remaining q pair0, all pair1 k/q, v pair1 halves
            add_proj(19, 0, 2)          # q0n2, needed slot 32
            add_proj(23, 3, 0)          # k1n0, needed slot 64
            for tt in range(5):
                add(26 + tt, v_half(tt, 1))
            add_proj(35, 0, 3)          # q0n3, needed slot 48
            add_proj(39, 3, 1)          # k1n1
            for tt in range(5, 10):
                add(43 + (tt - 5), v_half(tt, 1))
            add_proj(51, 3, 2)          # k1n2
            add_proj(55, 3, 3)          # k1n3
            add_proj(59, 1, 0)          # q1n0, needed slot 64
            for tt in range(10, 13):
                add(61 + (tt - 10), v_half(tt, 1))
            for tt in range(13, 16):
                add(67 + (tt - 13), v_half(tt, 1))
            add_proj(71, 1, 1)          # q1n1, needed slot 80
            add_proj(83, 1, 2)          # q1n2, needed slot 96
            add_proj(99, 1, 3)          # q1n3, needed slot 112

            # oproj for qb 0,1,2 embedded in blocks 5,6,7 (1 piece/slot,
            # PSUM double-buffered via ps_vm/ps_qk alternation)
            for blk, qb in ((5, 0), (6, 1), (7, 2)):
                st_o = {}
                for i in range(8):
                    def mk(q=qb, s=i // 2, d=i % 2, stx=st_o, alt=i % 2):
                        def run():
                            oproj_piece(q, s, d, stx, po_pool=(ps_qk if alt else ps_vm))
                        return run
                    add(16 * blk + 5 + i, mk())

            # ---- slot pipeline (av lags scores by 2 slots so AV never
            # blocks the next scores pair on a v-tile eviction) ----
            blocks = [(0, 0), (0, 1), (0, 2), (0, 3), (1, 0), (1, 1), (1, 2), (1, 3)]
            units = [make_block(p, qb) for (p, qb) in blocks]
            pend = []
            for s in range(128):
                bi, kt = s // 16, s % 16
                units[bi][0](kt)                       # sc_exp
                pend.append((bi, kt))
                if len(pend) > 2:
                    b2, k2 = pend.pop(0)
                    units[b2][1](k2)                   # av, lag 2
                    if k2 == 15:
                        units[b2][2]()                 # norm_drain
                if kt == 3 and bi > 0:
                    pprev, qprev = blocks[bi - 1]
                    stacked[(pprev, qprev)] = units[bi - 1][3]()   # norm_fin
                for fn in extras[s]:
                    fn()

            # ---- tail ----
            for b2, k2 in pend:
                units[b2][1](k2)
            units[7][2]()
            stacked[(1, 3)] = units[7][3]()
            st_o = {}
            tail_pools = [ps_vm, ps_qk, ps_av, ps_av]
            for i in range(8):
                oproj_piece(3, i // 2, i % 2, st_o,
                            evict=("scalar" if i % 2 else "vector"),
                            po_pool=tail_pools[i % 4])

            ctx.close()

    nc.compile()
    return nc


def _host_prep(hidden_states, w_qkv):
    """Per-core input maps (host-side shard + layout prep)."""
    invf = 1.0 / (THETA ** (np.arange(0, DH, 2, dtype=np.float32) / DH))
    t = np.arange(N, dtype=np.float32)
    d_idx = np.arange(128)
    f = invf[(d_idx % 64) // 2]
    ang = t[None, :] * f[:, None]
    cos2 = np.ascontiguousarray(np.cos(ang)).astype(BF)
    sign = np.where(d_idx % 2 == 0, -1.0, 1.0).astype(np.float32)
    sin2n = np.ascontiguousarray(np.sin(ang) * sign[:, None]).astype(BF)

    # [NT, 128, KT_D, 512] partition-major so device DMAs are contiguous
    xT_b = [np.ascontiguousarray(
                hidden_states[b].T.astype(BF).reshape(D // 128, 128, N // 512, 512)
                .transpose(2, 1, 0, 3))
            for b in range(B)]

    in_maps = []
    for c in range(8):
        b, g = c // 4, c % 4
        heads = [4 * g, 4 * g + 1, 4 * g + 2, 4 * g + 3]
        cols = []
        for off in (0, 1024):
            for h in heads:
                cols.append(w_qkv[:, off + h * 64: off + (h + 1) * 64])
        wqk = np.ascontiguousarray(
            np.concatenate(cols, axis=1).astype(BF).reshape(D // 128, 128, 512).transpose(1, 0, 2))
        # wv pair-major: [128, 2, KT_D, 128]
        wv = np.ascontiguousarray(
            np.concatenate([w_qkv[:, 2048 + h * 64: 2048 + (h + 1) * 64] for h in heads],
                           axis=1).astype(BF).reshape(D // 128, 128, 2, 128)
            .transpose(1, 2, 0, 3))
        in_maps.append({
            "xT": xT_b[b],
            "wqk": np.ascontiguousarray(wqk),
            "wv": np.ascontiguousarray(wv),
            "cos2": cos2,
            "sin2n": sin2n,
        })
    return in_maps


def kernel(hidden_states, w_qkv, w_out, b_out, _trace=False, _tmpdir=None):
    hidden_states = np.asarray(hidden_states, dtype=np.float32)
    w_qkv = np.asarray(w_qkv, dtype=np.float32)
    w_out = np.asarray(w_out, dtype=np.float32)
    b_out = np.asarray(b_out, dtype=np.float32)

    from concourse.bass_utils import run_bass_kernel_spmd

    if "nc" not in _CACHE:
        _CACHE["nc"] = _build()
    nc = _CACHE["nc"]

    in_maps = _host_prep(hidden_states, w_qkv)
    for c in range(8):
        g = c % 4
        wo = np.ascontiguousarray(
            w_out[4 * g * 64: 4 * g * 64 + 256, :].astype(BF).reshape(2, 128, 1024).transpose(1, 0, 2))
        in_maps[c]["wo"] = wo

    kwargs = {}
    if _trace:
        kwargs = dict(trace=True, tmpdir=_tmpdir)
    res = run_bass_kernel_spmd(nc, in_maps, core_ids=list(range(8)), **kwargs)

    out = np.zeros((B, N, D), dtype=np.float32)
    for c in range(8):
        out[c // 4] += res.results[c]["out"].reshape(N, D).astype(np.float32)
    out += b_out[None, None, :]
    if _trace:
        _CACHE["last_res"] = res
    return out
